# revision 1
# baseline (speedup 1.0000x reference)
"""DRL4TSP pointer-network decode on 8 Trainium2 NeuronCores.

Data-parallel over batch (16 items/core, 2 pipelined groups of 8).
All parameters replicated; the 64-step greedy decode runs fully on-device.

Key structure (per core, fp32 throughout):
  - Hoisted loop-invariants (computed on device by PE):
      U    = W_as@static_h + W_ad@dynamic_h      [H,(b,s)]
      V    = P_s@static_h                        [H,(b,s)]
      PST  = (P_c@static_h) transposed per item  [S,(b,H)]
      Gtab = (gru_wih@decoder_w)@static + biases [H,(gate,b,s)]
  - Per decode step: gather gi from Gtab by prev argmax (indirect_copy,
    wrapped per-16-partition semantics), GRU cell (sigmoid via tanh so the
    whole loop stays in one ACT table set), attention tanh + v-dot,
    softmax-free context fold (P_c@context = PS_T @ exp(l) / Z via PE),
    pointer tanh + v-dot, per-item argmax via max8/max_index on an
    item-major [8,64] psum produced by 8 accumulating "v in column b"
    matmuls, logp = -ln(sum exp(l-max)) with the Ln batched after the loop.
"""

import numpy as np


def _ensure_path():
    import sys

    try:
        import concourse.bass  # noqa: F401
        return
    except ImportError:
        pass
    for p in ("/opt/trn_rl_repo", "/root/.axon_site/_ro/trn_rl_repo"):
        if p not in sys.path:
            sys.path.insert(0, p)
    import concourse.bass  # noqa: F401


B, S, H = 128, 64, 128
NCORES = 8
BL = B // NCORES          # 16 items per core
NG = 2                    # groups per core
GB = BL // NG             # 8 items per group
W = GB * S                # 512 free width per group
F32 = "float32"

# constant-pack layout: name -> (col_offset, width); all in one [128, N] f32
_CP_WIDTHS = [
    ("st", BL * S), ("dy", BL * S), ("swT", H), ("dwT", H), ("w2T", 3 * H),
    ("wasT", H), ("wadT", H), ("wpsT", H), ("wpcT", H), ("wrT", H),
    ("whhT", 3 * H), ("vdA", 8 * GB), ("vdP", 8 * GB), ("w8", H),
    ("ones64", H), ("vecs", 8), ("biasrow", 5 * H), ("ones_row", W),
    ("base2", 2), ("ident", H),
]
CPACK_LAYOUT = {}
_c = 0
for _n, _w in _CP_WIDTHS:
    CPACK_LAYOUT[_n] = (_c, _w)
    _c += _w
CPACK_COLS = _c

_CACHE: dict = {}


def _build_program(n_steps: int = S):
    _ensure_path()
    import concourse.bass as bass
    import concourse.bacc as bacc
    import concourse.mybir as mybir
    from concourse.tile import TileContext

    dt = mybir.dt
    AF = mybir.ActivationFunctionType
    ALU = mybir.AluOpType
    AX = mybir.AxisListType

    nc = bacc.Bacc("TRN2", target_bir_lowering=False, debug=False,
                   enable_asserts=False, num_devices=NCORES)

    # ---------------- DRAM I/O ----------------
    def din(name, shape, d=dt.float32):
        return nc.dram_tensor(name, shape, d, kind="ExternalInput").ap()

    # All constants packed in one DRAM tensor -> one DMA -> one semaphore
    # (a matmul whose operands arrive on two DMA queues would need 2 sync
    #  waits; the LDWEIGHTS instruction only supports 1).
    cpack = din("cpack", [H, CPACK_COLS])

    out_idx = nc.dram_tensor("out_idx", [BL, S], dt.int32, kind="ExternalOutput").ap()
    out_logp = nc.dram_tensor("out_logp", [BL, S], dt.float32, kind="ExternalOutput").ap()

    with TileContext(nc) as tc:
        import contextlib

        ctx = contextlib.ExitStack()
        with ctx:
            cpool = ctx.enter_context(tc.tile_pool(name="consts", bufs=1))
            spool = ctx.enter_context(tc.tile_pool(name="work", bufs=3))
            gpool = ctx.enter_context(tc.tile_pool(name="gru", bufs=3))
            ppool_big = ctx.enter_context(
                tc.tile_pool(name="psbig", bufs=3, space="PSUM"))
            ppool_gh = ctx.enter_context(
                tc.tile_pool(name="psgh", bufs=2, space="PSUM"))
            ppool_sm = ctx.enter_context(
                tc.tile_pool(name="pssm", bufs=3, space="PSUM"))

            # ---- load all constants with one DMA ----
            cp_s = cpool.tile([H, CPACK_COLS], dt.float32, tag="cp", name="cp")
            nc.sync.dma_start(cp_s[:], cpack)

            def cslice(name, nrows):
                c0, w_ = CPACK_LAYOUT[name]
                return cp_s[0:nrows, c0:c0 + w_]

            st_s = cslice("st", 2)
            dy_s = cslice("dy", 2)
            swT_s = cslice("swT", 2)
            dwT_s = cslice("dwT", 2)
            w2T_s = cslice("w2T", 2)
            wasT_s = cslice("wasT", H)
            wadT_s = cslice("wadT", H)
            wpsT_s = cslice("wpsT", H)
            wpcT_s = cslice("wpcT", H)
            wrT_s = cslice("wrT", H)
            whhT_s = cslice("whhT", H)
            vdA_s = cslice("vdA", H)
            vdP_s = cslice("vdP", H)
            w8_s = cslice("w8", GB)
            ones64_s = cslice("ones64", S)
            vecs_s = cslice("vecs", H)
            biasrow_s = cslice("biasrow", 1)
            ones_s = cslice("ones_row", 1)
            base2_s = cslice("base2", H)

            ident_s = cslice("ident", H)

            # ---- persistent state ----
            h_s = cpool.tile([H, BL], dt.float32, tag="h", name="h")
            nc.vector.memset(h_s[:], 0.0)

            U_s = [cpool.tile([H, W], dt.float32, tag=f"U{g}", name=f"U{g}") for g in range(NG)]
            V_s = [cpool.tile([H, W], dt.float32, tag=f"V{g}", name=f"V{g}") for g in range(NG)]
            PST_s = [cpool.tile([S, GB * H], dt.float32, tag=f"PST{g}", name=f"PST{g}")
                     for g in range(NG)]
            Gt_s = [cpool.tile([H, 3 * W], dt.float32, tag=f"Gt{g}", name=f"Gt{g}")
                    for g in range(NG)]
            iu_s = [cpool.tile([H, 2], dt.int16, tag=f"iu{g}", name=f"iu{g}") for g in range(NG)]
            Z2b_s = [cpool.tile([GB, S], dt.float32, tag=f"Z2b{g}", name=f"Z2b{g}")
                     for g in range(NG)]
            oi_s = [cpool.tile([GB, S], dt.int32, tag=f"oi{g}", name=f"oi{g}") for g in range(NG)]

            # ---------------- precompute ----------------
            def colrange(g):
                return slice(g * W, (g + 1) * W)

            sh_s, dh_s = [], []
            for g in range(NG):
                cs = colrange(g)
                # static_h
                ps = ppool_big.tile([H, W], dt.float32, tag="pc", name="pc")
                nc.tensor.matmul(ps[:], swT_s[:], st_s[:, cs], start=True, stop=False)
                nc.tensor.matmul(ps[:], biasrow_s[:, 0:H], ones_s[:],
                                 start=False, stop=True)
                sh = cpool.tile([H, W], dt.float32, tag=f"sh{g}", name=f"sh{g}")
                nc.scalar.copy(sh[:], ps[:])
                sh_s.append(sh)
                # dynamic_h
                pd = ppool_big.tile([H, W], dt.float32, tag="pc", name="pc")
                nc.tensor.matmul(pd[:], dwT_s[:], dy_s[:, cs], start=True, stop=False)
                nc.tensor.matmul(pd[:], biasrow_s[:, H:2 * H], ones_s[:],
                                 start=False, stop=True)
                dh = cpool.tile([H, W], dt.float32, tag=f"dh{g}", name=f"dh{g}")
                nc.scalar.copy(dh[:], pd[:])
                dh_s.append(dh)

            for g in range(NG):
                cs = colrange(g)
                # U = W_as@sh + W_ad@dh
                pu = ppool_big.tile([H, W], dt.float32, tag="pc", name="pc")
                nc.tensor.matmul(pu[:], wasT_s[:], sh_s[g][:], start=True, stop=False)
                nc.tensor.matmul(pu[:], wadT_s[:], dh_s[g][:], start=False, stop=True)
                nc.scalar.copy(U_s[g][:], pu[:])
                # V = P_s@sh
                pv = ppool_big.tile([H, W], dt.float32, tag="pc", name="pc")
                nc.tensor.matmul(pv[:], wpsT_s[:], sh_s[g][:], start=True, stop=True)
                nc.scalar.copy(V_s[g][:], pv[:])
                # PS = P_c@sh -> transpose per item into PST
                pp = ppool_big.tile([H, W], dt.float32, tag="pc", name="pc")
                nc.tensor.matmul(pp[:], wpcT_s[:], sh_s[g][:], start=True, stop=True)
                ps_sb = spool.tile([H, W], dt.float32, tag="ps_sb", name="ps_sb")
                nc.scalar.copy(ps_sb[:], pp[:])
                for b in range(GB):
                    pt = ppool_sm.tile([S, H], dt.float32, tag="sm", name="pst_t")
                    nc.tensor.transpose(pt[:], ps_sb[:, b * S:(b + 1) * S],
                                        ident_s[:])
                    nc.scalar.copy(PST_s[g][:, b * H:(b + 1) * H], pt[:])
                # Gtab per gate
                for k in range(3):
                    pg = ppool_big.tile([H, W], dt.float32, tag="pc", name="pc")
                    nc.tensor.matmul(pg[:], w2T_s[:, k * H:(k + 1) * H],
                                     st_s[:, cs], start=True, stop=False)
                    nc.tensor.matmul(pg[:], biasrow_s[:, (2 + k) * H:(3 + k) * H],
                                     ones_s[:], start=False, stop=True)
                    nc.scalar.copy(Gt_s[g][:, k * W:(k + 1) * W], pg[:])

            # ---------------- decode loop ----------------
            gcols = [slice(g * GB, (g + 1) * GB) for g in range(NG)]

            def step(t, g):
                cs = gcols[g]
                # --- gh = Whh @ h (3 gate blocks) ---
                psGH = ppool_gh.tile([H, 3 * GB], dt.float32, tag="gh", name="gh")
                for k in range(3):
                    nc.tensor.matmul(psGH[:, k * GB:(k + 1) * GB],
                                     whhT_s[:, k * H:(k + 1) * H], h_s[:, cs],
                                     start=True, stop=(k == 2),
                                     skip_group_check=True)
                # ghnb = gh_n + bhh_n  (off critical path)
                ghnb = gpool.tile([H, GB], dt.float32, tag="ghnb", name="ghnb")
                nc.scalar.activation(ghnb[:], psGH[:, 2 * GB:3 * GB], AF.Identity,
                                     bias=vecs_s[:, 0:1])
                # --- gi (gathered previous step, or gi0 at t=0) ---
                if t == 0:
                    gi_rz = vecs_s[:, 1:3].unsqueeze(2).broadcast_to([H, 2, GB])
                    gi_n = vecs_s[:, 3:4].broadcast_to([H, GB])
                else:
                    gi = gpool.tile([H, 4 * GB], dt.float32, tag="gi", name="gi")
                    nc.gpsimd.ap_gather(gi[:], Gt_s[g][:], iu_s[g][:],
                                        channels=H, num_elems=3 * W, d=1,
                                        num_idxs=4 * GB)
                    gi_rz = gi[:, 0:2 * GB].rearrange("p (k b) -> p k b", k=2)
                    gi_n = gi[:, 2 * GB:3 * GB]
                nc.tensor.matmul(
                    psGH[:, 0:2 * GB].rearrange("p (k b) -> p k b", k=2),
                    ident_s[:], gi_rz, start=False, stop=True,
                    skip_group_check=True)
                # --- gates (sigmoid(x) = .5 + .5*tanh(x/2)) ---
                th = gpool.tile([H, 2 * GB], dt.float32, tag="th", name="th")
                nc.scalar.activation(th[:], psGH[:, 0:2 * GB], AF.Tanh, scale=0.5)
                t1 = gpool.tile([H, GB], dt.float32, tag="t1", name="t1")
                nc.vector.scalar_tensor_tensor(t1[:], th[:, 0:GB], 1.0, ghnb[:],
                                               op0=ALU.add, op1=ALU.mult)
                na = gpool.tile([H, GB], dt.float32, tag="na", name="na")
                nc.vector.scalar_tensor_tensor(na[:], t1[:], 0.5, gi_n,
                                               op0=ALU.mult, op1=ALU.add)
                n_s = gpool.tile([H, GB], dt.float32, tag="n", name="n")
                nc.scalar.activation(n_s[:], na[:], AF.Tanh)
                d_s = gpool.tile([H, GB], dt.float32, tag="d", name="d")
                nc.vector.tensor_tensor(d_s[:], h_s[:, cs], n_s[:],
                                        op=ALU.subtract)
                s1 = gpool.tile([H, GB], dt.float32, tag="s1", name="s1")
                nc.vector.scalar_tensor_tensor(s1[:], th[:, GB:2 * GB], 1.0, d_s[:],
                                               op0=ALU.add, op1=ALU.mult)
                nc.vector.scalar_tensor_tensor(h_s[:, cs], s1[:], 0.5, n_s[:],
                                               op0=ALU.mult, op1=ALU.add)
                # --- attention ---
                psW = ppool_sm.tile([H, GB], dt.float32, tag="sm", name="sm")
                nc.tensor.matmul(psW[:], wrT_s[:], h_s[:, cs], start=True, stop=True)
                psA = ppool_big.tile([H, W], dt.float32, tag="pc", name="pc")
                nc.vector.tensor_tensor(
                    psA[:].rearrange("p (b s) -> p b s", s=S),
                    U_s[g][:].rearrange("p (b s) -> p b s", s=S),
                    psW[:].unsqueeze(2).broadcast_to([H, GB, S]),
                    op=ALU.add)
                ea = spool.tile([H, W], dt.float32, tag="ea", name="ea")
                nc.scalar.activation(ea[:], psA[:], AF.Tanh)
                # transposed attn logits: lhsT = e-slice (M=s), rhs = v
                psQT = ppool_sm.tile([S, GB], dt.float32, tag="sm", name="sm")
                for b in range(GB):
                    nc.tensor.matmul(psQT[:, b:b + 1],
                                     ea[:, b * S:(b + 1) * S],
                                     vecs_s[:, 4:5], start=True, stop=True)
                qT = spool.tile([S, GB], dt.float32, tag="qT", name="qT")
                nc.scalar.activation(qT[:], psQT[:], AF.Exp)
                # w2un[:,b] = PST_b @ qT[:,b] ; Z replicated via ones matmul
                psW2 = ppool_sm.tile([H, GB], dt.float32, tag="sm", name="sm")
                for b in range(GB):
                    nc.tensor.matmul(psW2[:, b:b + 1],
                                     PST_s[g][:, b * H:(b + 1) * H],
                                     qT[:, b:b + 1], start=True, stop=True)
                psZ = ppool_sm.tile([H, GB], dt.float32, tag="sm", name="sm")
                nc.tensor.matmul(psZ[:], ones64_s[:], qT[:], start=True, stop=True)
                rz_s = gpool.tile([H, GB], dt.float32, tag="rz", name="rz")
                nc.vector.reciprocal(rz_s[:], psZ[:])
                w2 = gpool.tile([H, GB], dt.float32, tag="w2", name="w2")
                nc.vector.tensor_tensor(w2[:], psW2[:], rz_s[:], op=ALU.mult)
                # --- pointer ---
                psP2 = ppool_big.tile([H, W], dt.float32, tag="pc", name="pc")
                nc.vector.tensor_tensor(
                    psP2[:].rearrange("p (b s) -> p b s", s=S),
                    V_s[g][:].rearrange("p (b s) -> p b s", s=S),
                    w2[:].unsqueeze(2).broadcast_to([H, GB, S]),
                    op=ALU.add)
                ep = spool.tile([H, W], dt.float32, tag="ep", name="ep")
                nc.scalar.activation(ep[:], psP2[:], AF.Tanh)
                psLT = ppool_sm.tile([S, GB], dt.float32, tag="sm", name="sm")
                for b in range(GB):
                    nc.tensor.matmul(psLT[:, b:b + 1],
                                     ep[:, b * S:(b + 1) * S],
                                     vecs_s[:, 5:6], start=True, stop=True)
                lTs = spool.tile([S, GB], dt.float32, tag="lTs", name="lTs")
                nc.scalar.copy(lTs[:], psLT[:])
                psI2 = ppool_sm.tile([GB, S], dt.float32, tag="sm", name="sm")
                nc.tensor.transpose(psI2[:], lTs[:], ident_s[0:S, 0:S])
                lP = spool.tile([GB, S], dt.float32, tag="lP", name="lP")
                nc.scalar.copy(lP[:], psI2[:])
                # --- argmax / outputs ---
                mx = gpool.tile([GB, 8], dt.float32, tag="mx", name="mx")
                nc.vector.max(mx[:], lP[:])
                mi = gpool.tile([GB, 8], dt.uint16, tag="mi", name="mi")
                nc.vector.max_index(mi[:], mx[:], lP[:])
                nc.vector.tensor_copy(oi_s[g][:, t:t + 1], mi[:, 0:1])
                nm = gpool.tile([GB, 1], dt.float32, tag="nm", name="nm")
                nc.vector.tensor_scalar_mul(nm[:], mx[:, 0:1], -1.0)
                junk = gpool.tile([GB, S], dt.float32, tag="junk", name="junk")
                nc.scalar.activation(junk[:], lP[:], AF.Exp, bias=nm[:],
                                     accum_out=Z2b_s[g][:, t:t + 1])
                if t < n_steps - 1:
                    pf = gpool.tile([GB, 1], dt.float32, tag="pf", name="pf")
                    nc.vector.tensor_copy(pf[:], mi[:, 0:1])
                    psPtr = ppool_sm.tile([H, 1], dt.float32, tag="sm", name="sm")
                    nc.tensor.matmul(psPtr[:], w8_s[:], pf[:], start=True, stop=True)
                    nc.vector.tensor_tensor(iu_s[g][:], base2_s[:],
                                            psPtr[:].broadcast_to([H, 2]),
                                            op=ALU.add)

            for t in range(n_steps):
                for g in range(NG):
                    step(t, g)

            # ---------------- epilogue ----------------
            ns = n_steps
            for g in range(NG):
                lnq = spool.tile([GB, S], dt.float32, tag="lnq", name="lnq")
                nc.scalar.activation(lnq[:, 0:ns], Z2b_s[g][:, 0:ns], AF.Ln)
                olp = spool.tile([GB, S], dt.float32, tag="olp", name="olp")
                nc.scalar.mul(olp[:, 0:ns], lnq[:, 0:ns], -1.0)
                nc.sync.dma_start(out_logp[g * GB:(g + 1) * GB, 0:ns],
                                  olp[:, 0:ns])
                nc.sync.dma_start(out_idx[g * GB:(g + 1) * GB, 0:ns],
                                  oi_s[g][:, 0:ns])

    nc.compile()
    _legalize_waits(nc)
    return nc


def _legalize_waits(nc):
    """Engine instruction structs carry a limited number of sync waits
    (LDWEIGHTS: 1; ACT/DVE/Pool structs are similarly tight). Move extra
    waits onto injected same-engine nops placed immediately before."""
    import concourse.mybir as mybir

    CAPPED = {mybir.EngineType.PE, mybir.EngineType.Activation,
              mybir.EngineType.DVE, mybir.EngineType.Pool}
    # snapshot all block instruction lists first (nop creation appends to
    # the current bb; reassignment below drops those stray tail copies)
    blocks = []
    for f in nc.m.functions:
        for blk in f.blocks:
            blocks.append((blk, list(blk.instructions)))
    n_fixed = 0
    final = []
    for blk, insts in blocks:
        out = []
        for i in insts:
            si = i.sync_info
            if (i.engine in CAPPED and si is not None and si.on_wait
                    and len(si.on_wait) > 1
                    and type(i).__name__ != "InstNop"):
                for wt in si.on_wait[:-1]:
                    nop = nc.engines[i.engine].nop().ins
                    nop.sync_info = mybir.SyncInfo(on_wait=[wt], on_update=[])
                    out.append(nop)
                    n_fixed += 1
                i.sync_info = mybir.SyncInfo(on_wait=[si.on_wait[-1]],
                                             on_update=si.on_update)
            out.append(i)
        final.append((blk, out))
    # overwrite every block; stray nop appends (eng.nop() adds to the
    # current bb) are dropped because they are absent from the final lists
    for blk, out in final:
        blk.instructions = out


def _host_prep(inputs):
    """Build per-core input maps (weight prepack + batch sharding)."""
    f32 = np.float32
    st = np.ascontiguousarray(inputs["static"], dtype=f32)    # [B,2,S]
    dy = np.ascontiguousarray(inputs["dynamic"], dtype=f32)
    x0 = np.asarray(inputs["x0"], dtype=f32)
    sw, sb = np.asarray(inputs["static_w"], f32), np.asarray(inputs["static_b"], f32)
    dw, db = np.asarray(inputs["dynamic_w"], f32), np.asarray(inputs["dynamic_b"], f32)
    decw, decb = np.asarray(inputs["decoder_w"], f32), np.asarray(inputs["decoder_b"], f32)
    wih, whh = np.asarray(inputs["gru_wih"], f32), np.asarray(inputs["gru_whh"], f32)
    bih, bhh = np.asarray(inputs["gru_bih"], f32), np.asarray(inputs["gru_bhh"], f32)
    av, aW = np.asarray(inputs["attn_v"], f32), np.asarray(inputs["attn_W"], f32)
    pv, pW = np.asarray(inputs["ptr_v"], f32), np.asarray(inputs["ptr_W"], f32)

    W2 = (wih @ decw).astype(f32)                  # [3H,2]
    gbias = (wih @ decb + bih).astype(f32)         # [3H]
    bias_r = (gbias[0:H] + bhh[0:H]).astype(f32)
    bias_z = (gbias[H:2 * H] + bhh[H:2 * H]).astype(f32)
    bias_n = gbias[2 * H:3 * H].astype(f32)
    bhh_n = bhh[2 * H:3 * H].astype(f32)
    gi0 = (W2 @ x0 + gbias).astype(f32)
    gi0 = gi0 + np.concatenate([bhh[0:2 * H], np.zeros(H, f32)])  # bias-fold like Gtab

    vecs = np.zeros((H, 8), f32)
    vecs[:, 0] = bhh_n
    vecs[:, 1] = gi0[0:H]
    vecs[:, 2] = gi0[H:2 * H]
    vecs[:, 3] = gi0[2 * H:3 * H]
    vecs[:, 4] = av
    vecs[:, 5] = pv

    biasrow = np.concatenate([sb, db, bias_r, bias_z, bias_n]).reshape(1, 5 * H)

    vdA = np.zeros((H, 8 * GB), f32)
    vdP = np.zeros((H, 8 * GB), f32)
    for b in range(GB):
        vdA[:, b * GB + b] = av
        vdP[:, b * GB + b] = pv

    w8 = np.zeros((GB, H), f32)
    for m in range(H):
        w8[m % GB, m] = 1.0

    base2 = np.zeros((H, 2), f32)
    for p in range(H):
        q = p % 16
        for j in range(2):
            i = q + 16 * j
            if i < 3 * GB:
                gate, b = i // GB, i % GB
                base2[p, j] = gate * W + b * S

    parts = {
        "swT": sw.T, "dwT": dw.T,
        "w2T": np.concatenate([W2[k * H:(k + 1) * H, :].T for k in range(3)],
                              axis=1),
        "wasT": aW[:, 0:H].T, "wadT": aW[:, H:2 * H].T,
        "wpsT": pW[:, 0:H].T, "wpcT": pW[:, H:2 * H].T,
        "wrT": aW[:, 2 * H:3 * H].T,
        "whhT": np.concatenate([whh[k * H:(k + 1) * H, :].T for k in range(3)],
                               axis=1),
        "vdA": vdA, "vdP": vdP, "w8": w8,
        "ones64": np.ones((S, H), f32),
        "vecs": vecs, "biasrow": biasrow,
        "ones_row": np.ones((1, W), f32),
        "base2": base2,
        "ident": np.eye(H, dtype=f32),
    }
    cpack = np.zeros((H, CPACK_COLS), f32)
    for nme, arr in parts.items():
        c0, w_ = CPACK_LAYOUT[nme]
        arr = np.asarray(arr, f32)
        cpack[0:arr.shape[0], c0:c0 + w_] = arr

    in_maps = []
    for c in range(NCORES):
        sl = slice(c * BL, (c + 1) * BL)
        cp = cpack.copy()
        c0, w_ = CPACK_LAYOUT["st"]
        cp[0:2, c0:c0 + w_] = st[sl].transpose(1, 0, 2).reshape(2, BL * S)
        c0, w_ = CPACK_LAYOUT["dy"]
        cp[0:2, c0:c0 + w_] = dy[sl].transpose(1, 0, 2).reshape(2, BL * S)
        in_maps.append({"cpack": cp})
    return in_maps


def kernel(**inputs):
    _ensure_path()
    from concourse import bass_utils

    if "nc" not in _CACHE:
        _CACHE["nc"] = _build_program()
    nc = _CACHE["nc"]

    in_maps = _host_prep(inputs)
    res = bass_utils.run_bass_kernel_spmd(nc, in_maps, core_ids=list(range(NCORES)))
    ptrs = np.concatenate([r["out_idx"] for r in res.results], axis=0)
    logps = np.concatenate([r["out_logp"] for r in res.results], axis=0)
    return ptrs.astype(np.int32), logps.astype(np.float32)



# revision 4
# speedup vs baseline: 1.0010x; 1.0010x over previous
"""DRL4TSP pointer-network decode on 8 Trainium2 NeuronCores.

Data-parallel over batch (16 items/core, 2 pipelined groups of 8).
All parameters replicated; the 64-step greedy decode runs fully on-device.

Structure (per core, fp32 throughout):
  - Hoisted loop-invariants (computed on device by PE):
      U    = W_as@static_h + W_ad@dynamic_h      [H,(b,s)]
      V    = P_s@static_h                        [H,(b,s)]
      PST  = (P_c@static_h) transposed per item  [S,(b,H)]
      Gt   = (gru_wih@decoder_w)@static + biases [H,(gate,b,s)]
      GtT  = Gt transposed per (gate,item)       [S,(gate,b,H)]
  - Per decode step (serial chain kept as short as possible):
      GRU from psum GHQ (= whh@h + gi, accumulated by PE off-chain),
      attention tanh + v-dot, softmax-free context fold, pointer tanh
      + v-dot into a [S,(b)] psum, per-item argmax via gpsimd
      partition_all_reduce(max) + DVE is_equal one-hot, next gi via
      one-hot matmuls against GtT (bit-exact gather), ptr index via
      one-hot @ iota matmul. logp = max - ln(sum exp(l)), with exp
      row-sums banked per step and one Ln at the end.
"""

import numpy as np


def _ensure_path():
    import sys

    try:
        import concourse.bass  # noqa: F401
        return
    except ImportError:
        pass
    for p in ("/opt/trn_rl_repo", "/root/.axon_site/_ro/trn_rl_repo"):
        if p not in sys.path:
            sys.path.insert(0, p)
    import concourse.bass  # noqa: F401


B, S, H = 128, 64, 128
NCORES = 8
BL = B // NCORES          # 16 items per core
NG = 2                    # groups per core
GB = BL // NG             # 8 items per group
W = GB * S                # 512 free width per group
F32 = "float32"

# constant-pack layout: name -> (col_offset, width); all in one [128, N] f32
_CP_WIDTHS = [
    ("st", BL * S), ("dy", BL * S), ("swT", H), ("dwT", H), ("w2T", 3 * H),
    ("wasT", H), ("wadT", H), ("wpsT", H), ("wpcT", H), ("wrT", H),
    ("wrT05", H), ("whhT", 3 * H), ("whhn05T", H),
    ("ones64", H), ("vecs", 8), ("biasrow", 9 * H), ("ones_row", W),
    ("ident", H),
]
CPACK_LAYOUT = {}
_c = 0
for _n, _w in _CP_WIDTHS:
    CPACK_LAYOUT[_n] = (_c, _w)
    _c += _w
CPACK_COLS = _c

_CACHE: dict = {}


def _build_program(n_steps: int = S):
    _ensure_path()
    import concourse.bass as bass
    import concourse.bacc as bacc
    import concourse.mybir as mybir
    import concourse.bass_isa as bass_isa
    from concourse.tile import TileContext

    dt = mybir.dt
    AF = mybir.ActivationFunctionType
    ALU = mybir.AluOpType

    nc = bacc.Bacc("TRN2", target_bir_lowering=False, debug=False,
                   enable_asserts=False, num_devices=NCORES)

    # ---------------- DRAM I/O ----------------
    cpack = nc.dram_tensor("cpack", [H, CPACK_COLS], dt.float32,
                           kind="ExternalInput").ap()
    out_idx = nc.dram_tensor("out_idx", [BL, S], dt.int32,
                             kind="ExternalOutput").ap()
    out_logp = nc.dram_tensor("out_logp", [BL, S], dt.float32,
                              kind="ExternalOutput").ap()

    with TileContext(nc) as tc:
        import contextlib

        ctx = contextlib.ExitStack()
        with ctx:
            cpool = ctx.enter_context(tc.tile_pool(name="consts", bufs=1))
            spool = ctx.enter_context(tc.tile_pool(name="work", bufs=3))
            gpool = ctx.enter_context(tc.tile_pool(name="gru", bufs=3))
            ppool_big = ctx.enter_context(
                tc.tile_pool(name="psbig", bufs=3, space="PSUM"))
            ppool_gh = ctx.enter_context(
                tc.tile_pool(name="psgh", bufs=2, space="PSUM"))
            ppool_sm = ctx.enter_context(
                tc.tile_pool(name="pssm", bufs=3, space="PSUM"))

            # ---- load all constants with one DMA ----
            cp_s = cpool.tile([H, CPACK_COLS], dt.float32, tag="cp", name="cp")
            nc.sync.dma_start(cp_s[:], cpack)

            def cslice(name, nrows):
                c0, w_ = CPACK_LAYOUT[name]
                return cp_s[0:nrows, c0:c0 + w_]

            st_s = cslice("st", 2)
            dy_s = cslice("dy", 2)
            swT_s = cslice("swT", 2)
            dwT_s = cslice("dwT", 2)
            w2T_s = cslice("w2T", 2)
            wasT_s = cslice("wasT", H)
            wadT_s = cslice("wadT", H)
            wpsT_s = cslice("wpsT", H)
            wpcT_s = cslice("wpcT", H)
            wrT_s = cslice("wrT", H)
            wrT05_s = cslice("wrT05", H)
            whhT_s = cslice("whhT", H)
            whhn05T_s = cslice("whhn05T", H)
            ones64_s = cslice("ones64", S)
            vecs_s = cslice("vecs", H)
            biasrow_s = cslice("biasrow", 1)
            ones_s = cslice("ones_row", 1)
            ident_s = cslice("ident", H)

            # biasrow columns: [0:H]=static_b [H:2H]=dynamic_b
            #   [2H:5H]=Gtab gate biases (r,z incl bhh; n = gbias_n)
            #   [5H:8H]=gi0 rows (r,z incl bhh fold; n plain)
            #   [8H:9H]=0.5*bhh_n
            # vecs columns: 4=attn_v 5=ptr_v 6=iota64(rows 0:64)

            # ---- persistent state ----
            h_s = cpool.tile([H, BL], dt.float32, tag="h", name="h")
            nc.vector.memset(h_s[:], 0.0)

            U_s = [cpool.tile([H, W], dt.float32, tag=f"U{g}", name=f"U{g}")
                   for g in range(NG)]
            V_s = [cpool.tile([H, W], dt.float32, tag=f"V{g}", name=f"V{g}")
                   for g in range(NG)]
            PST_s = [cpool.tile([S, GB * H], dt.float32, tag=f"PST{g}",
                                name=f"PST{g}") for g in range(NG)]
            GtT_s = [cpool.tile([S, 3 * GB * H], dt.float32, tag=f"GtT{g}",
                                name=f"GtT{g}") for g in range(NG)]
            Zbuf_s = [cpool.tile([1, S * GB], dt.float32, tag=f"Zb{g}",
                                 name=f"Zb{g}") for g in range(NG)]
            mxbuf_s = [cpool.tile([1, S * GB], dt.float32, tag=f"mxb{g}",
                                  name=f"mxb{g}") for g in range(NG)]
            oi_s = [cpool.tile([GB, S], dt.int32, tag=f"oi{g}", name=f"oi{g}")
                    for g in range(NG)]

            # ---------------- precompute ----------------
            def colrange(g):
                return slice(g * W, (g + 1) * W)

            sh_s, dh_s = [], []
            for g in range(NG):
                cs = colrange(g)
                ps = ppool_big.tile([H, W], dt.float32, tag="pc", name="pc")
                nc.tensor.matmul(ps[:], swT_s[:], st_s[:, cs], start=True,
                                 stop=False)
                nc.tensor.matmul(ps[:], biasrow_s[:, 0:H], ones_s[:],
                                 start=False, stop=True)
                sh = cpool.tile([H, W], dt.float32, tag=f"sh{g}", name=f"sh{g}")
                nc.scalar.copy(sh[:], ps[:])
                sh_s.append(sh)
                pd = ppool_big.tile([H, W], dt.float32, tag="pc", name="pc")
                nc.tensor.matmul(pd[:], dwT_s[:], dy_s[:, cs], start=True,
                                 stop=False)
                nc.tensor.matmul(pd[:], biasrow_s[:, H:2 * H], ones_s[:],
                                 start=False, stop=True)
                dh = cpool.tile([H, W], dt.float32, tag=f"dh{g}", name=f"dh{g}")
                nc.scalar.copy(dh[:], pd[:])
                dh_s.append(dh)

            for g in range(NG):
                cs = colrange(g)
                # U = W_as@sh + W_ad@dh
                pu = ppool_big.tile([H, W], dt.float32, tag="pc", name="pc")
                nc.tensor.matmul(pu[:], wasT_s[:], sh_s[g][:], start=True,
                                 stop=False)
                nc.tensor.matmul(pu[:], wadT_s[:], dh_s[g][:], start=False,
                                 stop=True)
                nc.scalar.copy(U_s[g][:], pu[:])
                # V = P_s@sh
                pv = ppool_big.tile([H, W], dt.float32, tag="pc", name="pc")
                nc.tensor.matmul(pv[:], wpsT_s[:], sh_s[g][:], start=True,
                                 stop=True)
                nc.scalar.copy(V_s[g][:], pv[:])
                # PS = P_c@sh -> transpose per item into PST
                pp = ppool_big.tile([H, W], dt.float32, tag="pc", name="pc")
                nc.tensor.matmul(pp[:], wpcT_s[:], sh_s[g][:], start=True,
                                 stop=True)
                ps_sb = spool.tile([H, W], dt.float32, tag="ps_sb",
                                   name="ps_sb")
                nc.scalar.copy(ps_sb[:], pp[:])
                for b in range(GB):
                    pt = ppool_sm.tile([S, H], dt.float32, tag="sm",
                                       name="pst_t")
                    nc.tensor.transpose(pt[:], ps_sb[:, b * S:(b + 1) * S],
                                        ident_s[:])
                    nc.scalar.copy(PST_s[g][:, b * H:(b + 1) * H], pt[:])
                # Gtab per gate (with biases), then transpose per (gate,item)
                for k in range(3):
                    pg = ppool_big.tile([H, W], dt.float32, tag="pc", name="pc")
                    nc.tensor.matmul(pg[:], w2T_s[:, k * H:(k + 1) * H],
                                     st_s[:, cs], start=True, stop=False)
                    nc.tensor.matmul(pg[:], biasrow_s[:, (2 + k) * H:(3 + k) * H],
                                     ones_s[:], start=False, stop=True)
                    gt_sb = spool.tile([H, W], dt.float32, tag="gt_sb",
                                       name="gt_sb")
                    nc.scalar.copy(gt_sb[:], pg[:])
                    for b in range(GB):
                        pt = ppool_sm.tile([S, H], dt.float32, tag="sm",
                                           name="gt_t")
                        nc.tensor.transpose(pt[:], gt_sb[:, b * S:(b + 1) * S],
                                            ident_s[:])
                        dst = GtT_s[g][:, (k * GB + b) * H:(k * GB + b + 1) * H]
                        if b % 2 == 0:
                            nc.scalar.copy(dst, pt[:])
                        else:
                            nc.vector.tensor_copy(dst, pt[:])

            # ---------------- decode loop ----------------
            gcols = [slice(g * GB, (g + 1) * GB) for g in range(NG)]
            # per-group live tiles carried across phases
            psGHQ = [None, None]   # [H, 4*GB]: rz | NB | Q
            psW = [None, None]     # [H, GB]

            def gru_init(g):
                # psGHQ(0) = gi0 broadcast (+ 0.5*bhh_n in NB region)
                pg = ppool_gh.tile([H, 4 * GB], dt.float32, tag="ghq",
                                   name="ghq")
                for k in range(2):
                    nc.tensor.matmul(pg[:, k * GB:(k + 1) * GB],
                                     biasrow_s[:, (5 + k) * H:(6 + k) * H],
                                     ones_s[:, 0:GB], start=True, stop=True,
                                     skip_group_check=True)
                # NB = 0.5*bhh_n (no whh part at t=0: h0 = 0)
                nc.tensor.matmul(pg[:, 2 * GB:3 * GB],
                                 biasrow_s[:, 8 * H:9 * H],
                                 ones_s[:, 0:GB], start=True, stop=True,
                                 skip_group_check=True)
                # Q = gi0_n
                nc.tensor.matmul(pg[:, 3 * GB:4 * GB],
                                 biasrow_s[:, 7 * H:8 * H],
                                 ones_s[:, 0:GB], start=True, stop=True,
                                 skip_group_check=True)
                psGHQ[g] = pg

            def gru(t, g):
                """Consume psGHQ[g] (whh@h + gi), update h, emit psW for
                attention and (t < n_steps-1) whh-part of psGHQ(t+1)."""
                cs = gcols[g]
                pg = psGHQ[g]
                th = gpool.tile([H, 2 * GB], dt.float32, tag="th", name="th")
                nc.scalar.activation(th[:], pg[:, 0:2 * GB], AF.Tanh,
                                     scale=0.5)
                # r*ghnb' = (th_r+1) * NB   (NB = 0.5*(whh_n@h + bhh_n))
                t1 = gpool.tile([H, GB], dt.float32, tag="t1", name="t1")
                nc.vector.scalar_tensor_tensor(t1[:], th[:, 0:GB], 1.0,
                                               pg[:, 2 * GB:3 * GB],
                                               op0=ALU.add, op1=ALU.mult)
                na = gpool.tile([H, GB], dt.float32, tag="na", name="na")
                nc.vector.tensor_tensor(na[:], t1[:], pg[:, 3 * GB:4 * GB],
                                        op=ALU.add)
                n_s = gpool.tile([H, GB], dt.float32, tag="n", name="n")
                nc.scalar.activation(n_s[:], na[:], AF.Tanh)
                # psW = wrT@h' = wrT@n + 0.5*wrT@s1  (h' off critical path)
                pw = ppool_sm.tile([H, GB], dt.float32, tag="sm", name="pw")
                nc.tensor.matmul(pw[:], wrT_s[:], n_s[:], start=True,
                                 stop=False, skip_group_check=True)
                d_s = gpool.tile([H, GB], dt.float32, tag="d", name="d")
                nc.vector.tensor_tensor(d_s[:], h_s[:, cs], n_s[:],
                                        op=ALU.subtract)
                s1 = gpool.tile([H, GB], dt.float32, tag="s1", name="s1")
                nc.vector.scalar_tensor_tensor(s1[:], th[:, GB:2 * GB], 1.0,
                                               d_s[:], op0=ALU.add,
                                               op1=ALU.mult)
                nc.tensor.matmul(pw[:], wrT05_s[:], s1[:], start=False,
                                 stop=True, skip_group_check=True)
                psW[g] = pw
                # h' = 0.5*s1 + n
                nc.vector.scalar_tensor_tensor(h_s[:, cs], s1[:], 0.5, n_s[:],
                                               op0=ALU.mult, op1=ALU.add)
                if t < n_steps - 1:
                    # whh part of psGHQ(t+1)
                    pg2 = ppool_gh.tile([H, 4 * GB], dt.float32, tag="ghq",
                                        name="ghq")
                    for k in range(2):
                        nc.tensor.matmul(pg2[:, k * GB:(k + 1) * GB],
                                         whhT_s[:, k * H:(k + 1) * H],
                                         h_s[:, cs], start=True, stop=False,
                                         skip_group_check=True)
                    nc.tensor.matmul(pg2[:, 2 * GB:3 * GB], whhn05T_s[:],
                                     h_s[:, cs], start=True, stop=False,
                                     skip_group_check=True)
                    nc.tensor.matmul(pg2[:, 2 * GB:3 * GB],
                                     biasrow_s[:, 8 * H:9 * H],
                                     ones_s[:, 0:GB], start=False, stop=True,
                                     skip_group_check=True)
                    psGHQ[g] = pg2
                else:
                    psGHQ[g] = None

            def attn_ptr(t, g):
                """Attention + pointer; leaves psLT (pointer logits [S,GB])
                in psum and returns it."""
                # psA = U + bcast(psW)
                pA = ppool_big.tile([H, W], dt.float32, tag="pc", name="pc")
                nc.vector.tensor_tensor(
                    pA[:].rearrange("p (b s) -> p b s", s=S),
                    U_s[g][:].rearrange("p (b s) -> p b s", s=S),
                    psW[g][:].unsqueeze(2).broadcast_to([H, GB, S]),
                    op=ALU.add)
                ea = spool.tile([H, W], dt.float32, tag="ea", name="ea")
                nc.scalar.activation(ea[:], pA[:], AF.Tanh)
                psQT = ppool_sm.tile([S, GB], dt.float32, tag="sm", name="qt")
                for b in range(GB):
                    nc.tensor.matmul(psQT[:, b:b + 1],
                                     ea[:, b * S:(b + 1) * S],
                                     vecs_s[:, 4:5], start=True, stop=True)
                qT = spool.tile([S, GB], dt.float32, tag="qT", name="qT")
                nc.scalar.activation(qT[:], psQT[:], AF.Exp)
                psW2 = ppool_sm.tile([H, GB], dt.float32, tag="sm", name="w2")
                for b in range(GB):
                    nc.tensor.matmul(psW2[:, b:b + 1],
                                     PST_s[g][:, b * H:(b + 1) * H],
                                     qT[:, b:b + 1], start=True, stop=True)
                psZ = ppool_sm.tile([H, GB], dt.float32, tag="sm", name="z")
                nc.tensor.matmul(psZ[:], ones64_s[:], qT[:], start=True,
                                 stop=True)
                rz_s = gpool.tile([H, GB], dt.float32, tag="rz", name="rz")
                nc.vector.reciprocal(rz_s[:], psZ[:])
                w2 = gpool.tile([H, GB], dt.float32, tag="w2", name="w2")
                nc.vector.tensor_tensor(w2[:], psW2[:], rz_s[:], op=ALU.mult)
                pP = ppool_big.tile([H, W], dt.float32, tag="pc", name="pc")
                nc.vector.tensor_tensor(
                    pP[:].rearrange("p (b s) -> p b s", s=S),
                    V_s[g][:].rearrange("p (b s) -> p b s", s=S),
                    w2[:].unsqueeze(2).broadcast_to([H, GB, S]),
                    op=ALU.add)
                ep = spool.tile([H, W], dt.float32, tag="ep", name="ep")
                nc.scalar.activation(ep[:], pP[:], AF.Tanh)
                psLT = ppool_sm.tile([S, GB], dt.float32, tag="sm", name="lt")
                for b in range(GB):
                    nc.tensor.matmul(psLT[:, b:b + 1],
                                     ep[:, b * S:(b + 1) * S],
                                     vecs_s[:, 5:6], start=True, stop=True)
                return psLT

            def argmax_gi(t, g, psLT):
                """Argmax via partition-max + one-hot; outputs oi/logp rows;
                gi matmuls into psGHQ(t+1)."""
                lTs = spool.tile([S, GB], dt.float32, tag="lTs", name="lTs")
                nc.scalar.copy(lTs[:], psLT[:])
                mxr = spool.tile([S, GB], dt.float32, tag="mxr", name="mxr")
                nc.gpsimd.partition_all_reduce(
                    mxr[:], lTs[:], channels=S,
                    reduce_op=bass_isa.ReduceOp.max)
                oh = spool.tile([S, GB], dt.float32, tag="oh", name="oh")
                nc.vector.tensor_tensor(oh[:], psLT[:], mxr[:],
                                        op=ALU.is_equal)
                # --- off-chain: logp bookkeeping + ptr index ---
                qP = spool.tile([S, GB], dt.float32, tag="qP", name="qP")
                nc.scalar.activation(qP[:], psLT[:], AF.Exp)
                psZr = ppool_sm.tile([1, GB], dt.float32, tag="sm", name="zr")
                nc.tensor.matmul(psZr[:], ones64_s[:, 0:1], qP[:], start=True,
                                 stop=True)
                nc.vector.tensor_copy(Zbuf_s[g][:, t * GB:(t + 1) * GB],
                                      psZr[:])
                nc.vector.tensor_copy(mxbuf_s[g][:, t * GB:(t + 1) * GB],
                                      mxr[0:1, :])
                psIC = ppool_sm.tile([GB, 1], dt.float32, tag="sm", name="ic")
                nc.tensor.matmul(psIC[:], oh[:], vecs_s[0:S, 6:7], start=True,
                                 stop=True)
                nc.vector.tensor_copy(oi_s[g][:, t:t + 1], psIC[:])
                # --- gi matmuls into psGHQ(t+1) ---
                if t < n_steps - 1:
                    pg2 = psGHQ[g]
                    for k in range(2):
                        for b in range(GB):
                            nc.tensor.matmul(
                                pg2[:, k * GB + b:k * GB + b + 1],
                                GtT_s[g][:, (k * GB + b) * H:(k * GB + b + 1) * H],
                                oh[:, b:b + 1], start=False,
                                stop=(k == 1 and b == GB - 1),
                                skip_group_check=True)
                    for b in range(GB):
                        nc.tensor.matmul(
                            pg2[:, 3 * GB + b:3 * GB + b + 1],
                            GtT_s[g][:, (2 * GB + b) * H:(2 * GB + b + 1) * H],
                            oh[:, b:b + 1], start=True, stop=(b == GB - 1),
                            skip_group_check=True)

            for g in range(NG):
                gru_init(g)
            for t in range(n_steps):
                for g in range(NG):
                    gru(t, g)
                    psLT = attn_ptr(t, g)
                    argmax_gi(t, g, psLT)

            # ---------------- epilogue ----------------
            ns = n_steps
            for g in range(NG):
                lnq = spool.tile([1, S * GB], dt.float32, tag="lnq",
                                 name="lnq")
                nc.scalar.activation(lnq[:, 0:ns * GB],
                                     Zbuf_s[g][:, 0:ns * GB], AF.Ln)
                olp = spool.tile([1, S * GB], dt.float32, tag="olp",
                                 name="olp")
                nc.vector.tensor_tensor(olp[:, 0:ns * GB],
                                        mxbuf_s[g][:, 0:ns * GB],
                                        lnq[:, 0:ns * GB], op=ALU.subtract)
                # olp free order is (t, b); DRAM wants [b, t]
                olp3 = olp[:, 0:ns * GB].rearrange("p (t b) -> p b t", b=GB)
                for b in range(GB):
                    nc.sync.dma_start(
                        out_logp[g * GB + b:g * GB + b + 1, 0:ns],
                        olp3[:, b, :])
                nc.sync.dma_start(out_idx[g * GB:(g + 1) * GB, 0:ns],
                                  oi_s[g][:, 0:ns])

    nc.compile()
    _legalize_waits(nc)
    return nc


def _legalize_waits(nc):
    """Engine instruction structs carry a limited number of sync waits
    (LDWEIGHTS: 1; ACT/DVE/Pool structs are similarly tight). Move extra
    waits onto injected same-engine nops placed immediately before."""
    import concourse.mybir as mybir

    CAPPED = {mybir.EngineType.PE, mybir.EngineType.Activation,
              mybir.EngineType.DVE, mybir.EngineType.Pool}
    blocks = []
    for f in nc.m.functions:
        for blk in f.blocks:
            blocks.append((blk, list(blk.instructions)))
    final = []
    for blk, insts in blocks:
        out = []
        for i in insts:
            si = i.sync_info
            if (i.engine in CAPPED and si is not None and si.on_wait
                    and len(si.on_wait) > 1
                    and type(i).__name__ != "InstNop"):
                for wt in si.on_wait[:-1]:
                    nop = nc.engines[i.engine].nop().ins
                    nop.sync_info = mybir.SyncInfo(on_wait=[wt], on_update=[])
                    out.append(nop)
                i.sync_info = mybir.SyncInfo(on_wait=[si.on_wait[-1]],
                                             on_update=si.on_update)
            out.append(i)
        final.append((blk, out))
    for blk, out in final:
        blk.instructions = out


def _host_prep(inputs):
    """Build per-core input maps (weight prepack + batch sharding)."""
    f32 = np.float32
    st = np.ascontiguousarray(inputs["static"], dtype=f32)    # [B,2,S]
    dy = np.ascontiguousarray(inputs["dynamic"], dtype=f32)
    x0 = np.asarray(inputs["x0"], dtype=f32)
    sw, sb = np.asarray(inputs["static_w"], f32), np.asarray(inputs["static_b"], f32)
    dw, db = np.asarray(inputs["dynamic_w"], f32), np.asarray(inputs["dynamic_b"], f32)
    decw, decb = np.asarray(inputs["decoder_w"], f32), np.asarray(inputs["decoder_b"], f32)
    wih, whh = np.asarray(inputs["gru_wih"], f32), np.asarray(inputs["gru_whh"], f32)
    bih, bhh = np.asarray(inputs["gru_bih"], f32), np.asarray(inputs["gru_bhh"], f32)
    av, aW = np.asarray(inputs["attn_v"], f32), np.asarray(inputs["attn_W"], f32)
    pv, pW = np.asarray(inputs["ptr_v"], f32), np.asarray(inputs["ptr_W"], f32)

    W2 = (wih @ decw).astype(f32)                  # [3H,2]
    gbias = (wih @ decb + bih).astype(f32)         # [3H]
    bias_r = (gbias[0:H] + bhh[0:H]).astype(f32)
    bias_z = (gbias[H:2 * H] + bhh[H:2 * H]).astype(f32)
    bias_n = gbias[2 * H:3 * H].astype(f32)
    bhh_n = bhh[2 * H:3 * H].astype(f32)
    gi0 = (W2 @ x0 + gbias).astype(f32)
    gi0 = gi0 + np.concatenate([bhh[0:2 * H], np.zeros(H, f32)])

    vecs = np.zeros((H, 8), f32)
    vecs[:, 4] = av
    vecs[:, 5] = pv
    vecs[0:S, 6] = np.arange(S, dtype=f32)

    biasrow = np.concatenate(
        [sb, db, bias_r, bias_z, bias_n, gi0, 0.5 * bhh_n]).reshape(1, 9 * H)

    parts = {
        "swT": sw.T, "dwT": dw.T,
        "w2T": np.concatenate([W2[k * H:(k + 1) * H, :].T for k in range(3)],
                              axis=1),
        "wasT": aW[:, 0:H].T, "wadT": aW[:, H:2 * H].T,
        "wpsT": pW[:, 0:H].T, "wpcT": pW[:, H:2 * H].T,
        "wrT": aW[:, 2 * H:3 * H].T,
        "wrT05": 0.5 * aW[:, 2 * H:3 * H].T,
        "whhT": np.concatenate([whh[k * H:(k + 1) * H, :].T for k in range(3)],
                               axis=1),
        "whhn05T": 0.5 * whh[2 * H:3 * H, :].T,
        "ones64": np.ones((S, H), f32),
        "vecs": vecs, "biasrow": biasrow,
        "ones_row": np.ones((1, W), f32),
        "ident": np.eye(H, dtype=f32),
    }
    cpack = np.zeros((H, CPACK_COLS), f32)
    for nme, arr in parts.items():
        c0, w_ = CPACK_LAYOUT[nme]
        arr = np.asarray(arr, f32)
        cpack[0:arr.shape[0], c0:c0 + w_] = arr

    in_maps = []
    for c in range(NCORES):
        sl = slice(c * BL, (c + 1) * BL)
        cp = cpack.copy()
        c0, w_ = CPACK_LAYOUT["st"]
        cp[0:2, c0:c0 + w_] = st[sl].transpose(1, 0, 2).reshape(2, BL * S)
        c0, w_ = CPACK_LAYOUT["dy"]
        cp[0:2, c0:c0 + w_] = dy[sl].transpose(1, 0, 2).reshape(2, BL * S)
        in_maps.append({"cpack": cp})
    return in_maps


def kernel(**inputs):
    _ensure_path()
    from concourse import bass_utils

    if "nc" not in _CACHE:
        _CACHE["nc"] = _build_program()
    nc = _CACHE["nc"]

    in_maps = _host_prep(inputs)
    res = bass_utils.run_bass_kernel_spmd(nc, in_maps, core_ids=list(range(NCORES)))
    ptrs = np.concatenate([r["out_idx"] for r in res.results], axis=0)
    logps = np.concatenate([r["out_logp"] for r in res.results], axis=0)
    return ptrs.astype(np.int32), logps.astype(np.float32)


# revision 6
# speedup vs baseline: 1.3973x; 1.3959x over previous
"""DRL4TSP pointer-network decode on 8 Trainium2 NeuronCores.

Data-parallel over batch (16 items/core, 2 software-pipelined groups of 8).
All parameters replicated; the 64-step greedy decode runs fully on-device.

Structure (per core, fp32 throughout):
  - Hoisted loop-invariants (computed on device by PE):
      U    = W_as@static_h + W_ad@dynamic_h      [H,(b,s)]
      V    = P_s@static_h                        [H,(b,s)]
      PST  = (P_c@static_h) transposed per item  [S,(b,H)]
      GtT  = ((gru_wih@decoder_w)@static + bias) transposed per
             (gate,item)                         [S,(gate,b,H)]
  - Per decode step, the serial chain is split into 4 phases
    (gru / attn-front / attn-back / argmax-tail) and the two groups are
    emitted software-pipelined so every engine's in-order stream always
    has ready work:
      argmax: pointer logits [S,(b)] psum -> gpsimd partition_all_reduce
      (max) -> DVE is_equal one-hot -> next gi via one-hot matmuls
      against GtT (bit-exact gather); ptr index via one-hot @ iota.
      logp = max - ln(sum exp(l)) banked per step, one Ln at the end.
"""

import numpy as np


def _ensure_path():
    import sys

    try:
        import concourse.bass  # noqa: F401
        return
    except ImportError:
        pass
    for p in ("/opt/trn_rl_repo", "/root/.axon_site/_ro/trn_rl_repo"):
        if p not in sys.path:
            sys.path.insert(0, p)
    import concourse.bass  # noqa: F401


B, S, H = 128, 64, 128
NCORES = 8
BL = B // NCORES          # 16 items per core
NG = 2                    # groups per core
GB = BL // NG             # 8 items per group
W = GB * S                # 512 free width per group
F32 = "float32"

# constant-pack layout: name -> (col_offset, width); all in one [128, N] f32
_CP_WIDTHS = [
    ("st", BL * S), ("dy", BL * S), ("swT", H), ("dwT", H), ("w2T", 3 * H),
    ("wasT", H), ("wadT", H), ("wpsT", H), ("wpcT", H), ("wrT", H),
    ("wrT05", H), ("whhT", 3 * H), ("whhn05T", H),
    ("ones64", H), ("vecs", 8), ("biasrow", 9 * H), ("ones_row", W),
    ("ident", H),
]
CPACK_LAYOUT = {}
_c = 0
for _n, _w in _CP_WIDTHS:
    CPACK_LAYOUT[_n] = (_c, _w)
    _c += _w
CPACK_COLS = _c

_CACHE: dict = {}


def _build_program(n_steps: int = S):
    _ensure_path()
    import concourse.bass as bass
    import concourse.bacc as bacc
    import concourse.mybir as mybir
    import concourse.bass_isa as bass_isa
    from concourse.tile import TileContext

    dt = mybir.dt
    AF = mybir.ActivationFunctionType
    ALU = mybir.AluOpType

    nc = bacc.Bacc("TRN2", target_bir_lowering=False, debug=False,
                   enable_asserts=False, num_devices=NCORES)

    # ---------------- DRAM I/O ----------------
    cpack = nc.dram_tensor("cpack", [H, CPACK_COLS], dt.float32,
                           kind="ExternalInput").ap()
    out_idx = nc.dram_tensor("out_idx", [BL, S], dt.int32,
                             kind="ExternalOutput").ap()
    out_logp = nc.dram_tensor("out_logp", [BL, S], dt.float32,
                              kind="ExternalOutput").ap()

    with TileContext(nc) as tc:
        import contextlib

        ctx = contextlib.ExitStack()
        with ctx:
            cpool = ctx.enter_context(tc.tile_pool(name="consts", bufs=1))
            spool = ctx.enter_context(tc.tile_pool(name="work", bufs=3))
            gpool = ctx.enter_context(tc.tile_pool(name="gru", bufs=3))
            ppool_big = ctx.enter_context(
                tc.tile_pool(name="psbig", bufs=3, space="PSUM"))
            ppool_gh = ctx.enter_context(
                tc.tile_pool(name="psgh", bufs=2, space="PSUM"))
            ppool_fix = ctx.enter_context(
                tc.tile_pool(name="psfix", bufs=1, space="PSUM"))

            # ---- load all constants with one DMA ----
            cp_s = cpool.tile([H, CPACK_COLS], dt.float32, tag="cp", name="cp")
            nc.sync.dma_start(cp_s[:], cpack)

            def cslice(name, nrows):
                c0, w_ = CPACK_LAYOUT[name]
                return cp_s[0:nrows, c0:c0 + w_]

            st_s = cslice("st", 2)
            dy_s = cslice("dy", 2)
            swT_s = cslice("swT", 2)
            dwT_s = cslice("dwT", 2)
            w2T_s = cslice("w2T", 2)
            wasT_s = cslice("wasT", H)
            wadT_s = cslice("wadT", H)
            wpsT_s = cslice("wpsT", H)
            wpcT_s = cslice("wpcT", H)
            wrT_s = cslice("wrT", H)
            wrT05_s = cslice("wrT05", H)
            whhT_s = cslice("whhT", H)
            whhn05T_s = cslice("whhn05T", H)
            ones64_s = cslice("ones64", S)
            vecs_s = cslice("vecs", H)
            biasrow_s = cslice("biasrow", 1)
            ones_s = cslice("ones_row", 1)
            ident_s = cslice("ident", H)

            # biasrow columns: [0:H]=static_b [H:2H]=dynamic_b
            #   [2H:5H]=Gtab gate biases (r,z incl bhh; n = gbias_n)
            #   [5H:8H]=gi0 rows (r,z incl bhh fold; n plain)
            #   [8H:9H]=0.5*bhh_n
            # vecs columns: 4=attn_v 5=ptr_v 6=iota64(rows 0:64)

            # ---- persistent state ----
            h_s = cpool.tile([H, BL], dt.float32, tag="h", name="h")
            nc.vector.memset(h_s[:], 0.0)

            U_s = [cpool.tile([H, W], dt.float32, tag=f"U{g}", name=f"U{g}")
                   for g in range(NG)]
            V_s = [cpool.tile([H, W], dt.float32, tag=f"V{g}", name=f"V{g}")
                   for g in range(NG)]
            PST_s = [cpool.tile([S, GB * H], dt.float32, tag=f"PST{g}",
                                name=f"PST{g}") for g in range(NG)]
            GtT_s = [cpool.tile([S, 3 * GB * H], dt.float32, tag=f"GtT{g}",
                                name=f"GtT{g}") for g in range(NG)]
            Zbuf_s = [cpool.tile([1, S * GB], dt.float32, tag=f"Zb{g}",
                                 name=f"Zb{g}") for g in range(NG)]
            mxbuf_s = [cpool.tile([1, S * GB], dt.float32, tag=f"mxb{g}",
                                  name=f"mxb{g}") for g in range(NG)]
            oi_s = [cpool.tile([GB, S], dt.int32, tag=f"oi{g}", name=f"oi{g}")
                    for g in range(NG)]

            # persistent per-group psum scratch (one full bank each):
            #   pw [H,0:8] | qt [0:64,8:16] | w2p [H,16:24] | z [H,24:32]
            #   lt [0:64,32:40] | zr [0:1,40:48] | ic [0:8,48:49]
            fix = [ppool_fix.tile([H, 512], dt.float32, tag=f"fix{g}",
                                  name=f"fix{g}") for g in range(NG)]
            pw_r = [fx[:, 0:GB] for fx in fix]
            qt_r = [fx[0:S, GB:2 * GB] for fx in fix]
            w2p_r = [fx[:, 2 * GB:3 * GB] for fx in fix]
            z_r = [fx[:, 3 * GB:4 * GB] for fx in fix]
            lt_r = [fx[0:S, 4 * GB:5 * GB] for fx in fix]
            zr_r = [fx[0:1, 5 * GB:6 * GB] for fx in fix]
            ic_r = [fx[0:GB, 6 * GB:6 * GB + 1] for fx in fix]

            # ---------------- precompute ----------------
            def colrange(g):
                return slice(g * W, (g + 1) * W)

            sh_s, dh_s = [], []
            for g in range(NG):
                cs = colrange(g)
                ps = ppool_big.tile([H, W], dt.float32, tag="pc", name="pc")
                nc.tensor.matmul(ps[:], swT_s[:], st_s[:, cs], start=True,
                                 stop=False)
                nc.tensor.matmul(ps[:], biasrow_s[:, 0:H], ones_s[:],
                                 start=False, stop=True)
                sh = cpool.tile([H, W], dt.float32, tag=f"sh{g}", name=f"sh{g}")
                nc.scalar.copy(sh[:], ps[:])
                sh_s.append(sh)
                pd = ppool_big.tile([H, W], dt.float32, tag="pc", name="pc")
                nc.tensor.matmul(pd[:], dwT_s[:], dy_s[:, cs], start=True,
                                 stop=False)
                nc.tensor.matmul(pd[:], biasrow_s[:, H:2 * H], ones_s[:],
                                 start=False, stop=True)
                dh = cpool.tile([H, W], dt.float32, tag=f"dh{g}", name=f"dh{g}")
                nc.scalar.copy(dh[:], pd[:])
                dh_s.append(dh)

            for g in range(NG):
                cs = colrange(g)
                # U = W_as@sh + W_ad@dh
                pu = ppool_big.tile([H, W], dt.float32, tag="pc", name="pc")
                nc.tensor.matmul(pu[:], wasT_s[:], sh_s[g][:], start=True,
                                 stop=False)
                nc.tensor.matmul(pu[:], wadT_s[:], dh_s[g][:], start=False,
                                 stop=True)
                nc.scalar.copy(U_s[g][:], pu[:])
                # V = P_s@sh
                pv = ppool_big.tile([H, W], dt.float32, tag="pc", name="pc")
                nc.tensor.matmul(pv[:], wpsT_s[:], sh_s[g][:], start=True,
                                 stop=True)
                nc.scalar.copy(V_s[g][:], pv[:])
                # PS = P_c@sh -> transpose per item into PST
                pp = ppool_big.tile([H, W], dt.float32, tag="pc", name="pc")
                nc.tensor.matmul(pp[:], wpcT_s[:], sh_s[g][:], start=True,
                                 stop=True)
                ps_sb = spool.tile([H, W], dt.float32, tag="ps_sb",
                                   name="ps_sb")
                nc.scalar.copy(ps_sb[:], pp[:])
                for b in range(GB):
                    pt = ppool_big.tile([S, H], dt.float32, tag="pc",
                                        name="pst_t")
                    nc.tensor.transpose(pt[:], ps_sb[:, b * S:(b + 1) * S],
                                        ident_s[:])
                    nc.scalar.copy(PST_s[g][:, b * H:(b + 1) * H], pt[:])
                # Gtab per gate (with biases), then transpose per (gate,item)
                for k in range(3):
                    pg = ppool_big.tile([H, W], dt.float32, tag="pc", name="pc")
                    nc.tensor.matmul(pg[:], w2T_s[:, k * H:(k + 1) * H],
                                     st_s[:, cs], start=True, stop=False)
                    nc.tensor.matmul(pg[:], biasrow_s[:, (2 + k) * H:(3 + k) * H],
                                     ones_s[:], start=False, stop=True)
                    gt_sb = spool.tile([H, W], dt.float32, tag="gt_sb",
                                       name="gt_sb")
                    nc.scalar.copy(gt_sb[:], pg[:])
                    for b in range(GB):
                        pt = ppool_big.tile([S, H], dt.float32, tag="pc",
                                            name="gt_t")
                        nc.tensor.transpose(pt[:], gt_sb[:, b * S:(b + 1) * S],
                                            ident_s[:])
                        dst = GtT_s[g][:, (k * GB + b) * H:(k * GB + b + 1) * H]
                        if b % 2 == 0:
                            nc.scalar.copy(dst, pt[:])
                        else:
                            nc.vector.tensor_copy(dst, pt[:])

            # ---------------- decode loop ----------------
            gcols = [slice(g * GB, (g + 1) * GB) for g in range(NG)]
            psGHQ = [None, None]   # [H, 4*GB]: rz | NB | Q
            oh_t = [None, None]
            lTs_t = [None, None]
            mxr_t = [None, None]

            def gru_init(g):
                pg = ppool_gh.tile([H, 4 * GB], dt.float32, tag="ghq",
                                   name="ghq")
                for k in range(2):
                    nc.tensor.matmul(pg[:, k * GB:(k + 1) * GB],
                                     biasrow_s[:, (5 + k) * H:(6 + k) * H],
                                     ones_s[:, 0:GB], start=True, stop=True,
                                     skip_group_check=True)
                nc.tensor.matmul(pg[:, 2 * GB:3 * GB],
                                 biasrow_s[:, 8 * H:9 * H],
                                 ones_s[:, 0:GB], start=True, stop=True,
                                 skip_group_check=True)
                nc.tensor.matmul(pg[:, 3 * GB:4 * GB],
                                 biasrow_s[:, 7 * H:8 * H],
                                 ones_s[:, 0:GB], start=True, stop=True,
                                 skip_group_check=True)
                psGHQ[g] = pg

            def gru(t, g):
                """psGHQ (whh@h + gi) -> gates -> h update; emits psW for
                this step's attention and whh part of psGHQ(t+1)."""
                cs = gcols[g]
                pg = psGHQ[g]
                th = gpool.tile([H, 2 * GB], dt.float32, tag="th", name="th")
                nc.scalar.activation(th[:], pg[:, 0:2 * GB], AF.Tanh,
                                     scale=0.5)
                t1 = gpool.tile([H, GB], dt.float32, tag="t1", name="t1")
                nc.vector.scalar_tensor_tensor(t1[:], th[:, 0:GB], 1.0,
                                               pg[:, 2 * GB:3 * GB],
                                               op0=ALU.add, op1=ALU.mult)
                na = gpool.tile([H, GB], dt.float32, tag="na", name="na")
                nc.vector.tensor_tensor(na[:], t1[:], pg[:, 3 * GB:4 * GB],
                                        op=ALU.add)
                n_s = gpool.tile([H, GB], dt.float32, tag="n", name="n")
                nc.scalar.activation(n_s[:], na[:], AF.Tanh)
                # psW = wrT@h' = wrT@n + 0.5*wrT@s1   (h' off critical path)
                nc.tensor.matmul(pw_r[g], wrT_s[:], n_s[:], start=True,
                                 stop=False, skip_group_check=True)
                d_s = gpool.tile([H, GB], dt.float32, tag="d", name="d")
                nc.vector.tensor_tensor(d_s[:], h_s[:, cs], n_s[:],
                                        op=ALU.subtract)
                s1 = gpool.tile([H, GB], dt.float32, tag="s1", name="s1")
                nc.vector.scalar_tensor_tensor(s1[:], th[:, GB:2 * GB], 1.0,
                                               d_s[:], op0=ALU.add,
                                               op1=ALU.mult)
                nc.tensor.matmul(pw_r[g], wrT05_s[:], s1[:], start=False,
                                 stop=True, skip_group_check=True)
                if t < n_steps - 1:
                    nc.vector.scalar_tensor_tensor(h_s[:, cs], s1[:], 0.5,
                                                   n_s[:], op0=ALU.mult,
                                                   op1=ALU.add)
                    pg2 = ppool_gh.tile([H, 4 * GB], dt.float32, tag="ghq",
                                        name="ghq")
                    for k in range(2):
                        nc.tensor.matmul(pg2[:, k * GB:(k + 1) * GB],
                                         whhT_s[:, k * H:(k + 1) * H],
                                         h_s[:, cs], start=True, stop=False,
                                         skip_group_check=True)
                    nc.tensor.matmul(pg2[:, 2 * GB:3 * GB], whhn05T_s[:],
                                     h_s[:, cs], start=True, stop=False,
                                     skip_group_check=True)
                    nc.tensor.matmul(pg2[:, 2 * GB:3 * GB],
                                     biasrow_s[:, 8 * H:9 * H],
                                     ones_s[:, 0:GB], start=False, stop=True,
                                     skip_group_check=True)
                    psGHQ[g] = pg2
                else:
                    psGHQ[g] = None

            def front(t, g):
                """Attention front: psA -> ea -> attn logits -> exp ->
                context matmuls."""
                pA = ppool_big.tile([H, W], dt.float32, tag="pc", name="pc")
                nc.vector.tensor_tensor(
                    pA[:].rearrange("p (b s) -> p b s", s=S),
                    U_s[g][:].rearrange("p (b s) -> p b s", s=S),
                    pw_r[g].unsqueeze(2).broadcast_to([H, GB, S]),
                    op=ALU.add)
                ea = spool.tile([H, W], dt.float32, tag="ea", name="ea")
                nc.scalar.activation(ea[:], pA[:], AF.Tanh)
                for b in range(GB):
                    nc.tensor.matmul(qt_r[g][:, b:b + 1],
                                     ea[:, b * S:(b + 1) * S],
                                     vecs_s[:, 4:5], start=True, stop=True,
                                     skip_group_check=True)
                qT = spool.tile([S, GB], dt.float32, tag="qT", name="qT")
                nc.scalar.activation(qT[:], qt_r[g], AF.Exp)
                for b in range(GB):
                    nc.tensor.matmul(w2p_r[g][:, b:b + 1],
                                     PST_s[g][:, b * H:(b + 1) * H],
                                     qT[:, b:b + 1], start=True, stop=True,
                                     skip_group_check=True)
                nc.tensor.matmul(z_r[g], ones64_s[:], qT[:], start=True,
                                 stop=True, skip_group_check=True)

            def back(t, g):
                """Attention back: softmax fold -> pointer tanh -> pointer
                logits -> partition max; qP/Z row for logp."""
                rz_s = gpool.tile([H, GB], dt.float32, tag="rz", name="rz")
                nc.vector.reciprocal(rz_s[:], z_r[g])
                w2 = gpool.tile([H, GB], dt.float32, tag="w2", name="w2")
                nc.vector.tensor_tensor(w2[:], w2p_r[g], rz_s[:], op=ALU.mult)
                pP = ppool_big.tile([H, W], dt.float32, tag="pc", name="pc")
                nc.vector.tensor_tensor(
                    pP[:].rearrange("p (b s) -> p b s", s=S),
                    V_s[g][:].rearrange("p (b s) -> p b s", s=S),
                    w2[:].unsqueeze(2).broadcast_to([H, GB, S]),
                    op=ALU.add)
                ep = spool.tile([H, W], dt.float32, tag="ep", name="ep")
                nc.scalar.activation(ep[:], pP[:], AF.Tanh)
                for b in range(GB):
                    nc.tensor.matmul(lt_r[g][:, b:b + 1],
                                     ep[:, b * S:(b + 1) * S],
                                     vecs_s[:, 5:6], start=True, stop=True,
                                     skip_group_check=True)
                lTs = spool.tile([S, GB], dt.float32, tag="lTs", name="lTs")
                nc.scalar.copy(lTs[:], lt_r[g])
                mxr = spool.tile([S, GB], dt.float32, tag="mxr", name="mxr")
                nc.gpsimd.partition_all_reduce(
                    mxr[:], lTs[:], channels=S,
                    reduce_op=bass_isa.ReduceOp.max)
                qP = spool.tile([S, GB], dt.float32, tag="qP", name="qP")
                nc.scalar.activation(qP[:], lt_r[g], AF.Exp)
                nc.tensor.matmul(zr_r[g], ones64_s[:, 0:1], qP[:], start=True,
                                 stop=True, skip_group_check=True)
                lTs_t[g] = lTs
                mxr_t[g] = mxr

            def tail(t, g):
                """Argmax one-hot; gi matmuls into psGHQ(t+1); oi/logp
                bookkeeping (all deps already satisfied here)."""
                oh = spool.tile([S, GB], dt.float32, tag="oh", name="oh")
                nc.vector.tensor_tensor(oh[:], lTs_t[g][:], mxr_t[g][:],
                                        op=ALU.is_equal)
                if t < n_steps - 1:
                    pg2 = psGHQ[g]
                    for k in range(2):
                        for b in range(GB):
                            nc.tensor.matmul(
                                pg2[:, k * GB + b:k * GB + b + 1],
                                GtT_s[g][:, (k * GB + b) * H:(k * GB + b + 1) * H],
                                oh[:, b:b + 1], start=False,
                                stop=(k == 1 and b == GB - 1),
                                skip_group_check=True)
                    for b in range(GB):
                        nc.tensor.matmul(
                            pg2[:, 3 * GB + b:3 * GB + b + 1],
                            GtT_s[g][:, (2 * GB + b) * H:(2 * GB + b + 1) * H],
                            oh[:, b:b + 1], start=(b == 0), stop=(b == GB - 1),
                            skip_group_check=True)
                nc.tensor.matmul(ic_r[g], oh[:], vecs_s[0:S, 6:7], start=True,
                                 stop=True, skip_group_check=True)
                nc.vector.tensor_copy(Zbuf_s[g][:, t * GB:(t + 1) * GB],
                                      zr_r[g])
                nc.vector.tensor_copy(oi_s[g][:, t:t + 1], ic_r[g])
                nc.gpsimd.tensor_copy(mxbuf_s[g][:, t * GB:(t + 1) * GB],
                                      mxr_t[g][0:1, :])

            # software-pipelined emission:
            #   body(t) = tailA(t-1) gruA(t) backB(t-1) frontA(t)
            #             tailB(t-1) gruB(t) backA(t) frontB(t)
            for g in range(NG):
                gru_init(g)
            for t in range(n_steps):
                if t > 0:
                    tail(t - 1, 0)
                gru(t, 0)
                if t > 0:
                    back(t - 1, 1)
                front(t, 0)
                if t > 0:
                    tail(t - 1, 1)
                gru(t, 1)
                back(t, 0)
                front(t, 1)
            tail(n_steps - 1, 0)
            back(n_steps - 1, 1)
            tail(n_steps - 1, 1)

            # ---------------- epilogue ----------------
            ns = n_steps
            for g in range(NG):
                lnq = spool.tile([1, S * GB], dt.float32, tag="lnq",
                                 name="lnq")
                nc.scalar.activation(lnq[:, 0:ns * GB],
                                     Zbuf_s[g][:, 0:ns * GB], AF.Ln)
                olp = spool.tile([1, S * GB], dt.float32, tag="olp",
                                 name="olp")
                nc.vector.tensor_tensor(olp[:, 0:ns * GB],
                                        mxbuf_s[g][:, 0:ns * GB],
                                        lnq[:, 0:ns * GB], op=ALU.subtract)
                # olp free order is (t, b); DRAM wants [b, t]
                olp3 = olp[:, 0:ns * GB].rearrange("p (t b) -> p b t", b=GB)
                for b in range(GB):
                    nc.sync.dma_start(
                        out_logp[g * GB + b:g * GB + b + 1, 0:ns],
                        olp3[:, b, :])
                nc.sync.dma_start(out_idx[g * GB:(g + 1) * GB, 0:ns],
                                  oi_s[g][:, 0:ns])

    nc.compile()
    _legalize_waits(nc)
    return nc


def _legalize_waits(nc):
    """Engine instruction structs carry a limited number of sync waits
    (LDWEIGHTS: 1; ACT/DVE/Pool structs are similarly tight). Move extra
    waits onto injected same-engine nops placed immediately before."""
    import concourse.mybir as mybir

    CAPPED = {mybir.EngineType.PE, mybir.EngineType.Activation,
              mybir.EngineType.DVE, mybir.EngineType.Pool}
    blocks = []
    for f in nc.m.functions:
        for blk in f.blocks:
            blocks.append((blk, list(blk.instructions)))
    final = []
    for blk, insts in blocks:
        out = []
        for i in insts:
            si = i.sync_info
            if (i.engine in CAPPED and si is not None and si.on_wait
                    and len(si.on_wait) > 1
                    and type(i).__name__ != "InstNop"):
                for wt in si.on_wait[:-1]:
                    nop = nc.engines[i.engine].nop().ins
                    nop.sync_info = mybir.SyncInfo(on_wait=[wt], on_update=[])
                    out.append(nop)
                i.sync_info = mybir.SyncInfo(on_wait=[si.on_wait[-1]],
                                             on_update=si.on_update)
            out.append(i)
        final.append((blk, out))
    for blk, out in final:
        blk.instructions = out


def _host_prep(inputs):
    """Build per-core input maps (weight prepack + batch sharding)."""
    f32 = np.float32
    st = np.ascontiguousarray(inputs["static"], dtype=f32)    # [B,2,S]
    dy = np.ascontiguousarray(inputs["dynamic"], dtype=f32)
    x0 = np.asarray(inputs["x0"], dtype=f32)
    sw, sb = np.asarray(inputs["static_w"], f32), np.asarray(inputs["static_b"], f32)
    dw, db = np.asarray(inputs["dynamic_w"], f32), np.asarray(inputs["dynamic_b"], f32)
    decw, decb = np.asarray(inputs["decoder_w"], f32), np.asarray(inputs["decoder_b"], f32)
    wih, whh = np.asarray(inputs["gru_wih"], f32), np.asarray(inputs["gru_whh"], f32)
    bih, bhh = np.asarray(inputs["gru_bih"], f32), np.asarray(inputs["gru_bhh"], f32)
    av, aW = np.asarray(inputs["attn_v"], f32), np.asarray(inputs["attn_W"], f32)
    pv, pW = np.asarray(inputs["ptr_v"], f32), np.asarray(inputs["ptr_W"], f32)

    W2 = (wih @ decw).astype(f32)                  # [3H,2]
    gbias = (wih @ decb + bih).astype(f32)         # [3H]
    bias_r = (gbias[0:H] + bhh[0:H]).astype(f32)
    bias_z = (gbias[H:2 * H] + bhh[H:2 * H]).astype(f32)
    bias_n = gbias[2 * H:3 * H].astype(f32)
    bhh_n = bhh[2 * H:3 * H].astype(f32)
    gi0 = (W2 @ x0 + gbias).astype(f32)
    gi0 = gi0 + np.concatenate([bhh[0:2 * H], np.zeros(H, f32)])

    vecs = np.zeros((H, 8), f32)
    vecs[:, 4] = av
    vecs[:, 5] = pv
    vecs[0:S, 6] = np.arange(S, dtype=f32)

    biasrow = np.concatenate(
        [sb, db, bias_r, bias_z, bias_n, gi0, 0.5 * bhh_n]).reshape(1, 9 * H)

    parts = {
        "swT": sw.T, "dwT": dw.T,
        "w2T": np.concatenate([W2[k * H:(k + 1) * H, :].T for k in range(3)],
                              axis=1),
        "wasT": aW[:, 0:H].T, "wadT": aW[:, H:2 * H].T,
        "wpsT": pW[:, 0:H].T, "wpcT": pW[:, H:2 * H].T,
        "wrT": aW[:, 2 * H:3 * H].T,
        "wrT05": 0.5 * aW[:, 2 * H:3 * H].T,
        "whhT": np.concatenate([whh[k * H:(k + 1) * H, :].T for k in range(3)],
                               axis=1),
        "whhn05T": 0.5 * whh[2 * H:3 * H, :].T,
        "ones64": np.ones((S, H), f32),
        "vecs": vecs, "biasrow": biasrow,
        "ones_row": np.ones((1, W), f32),
        "ident": np.eye(H, dtype=f32),
    }
    cpack = np.zeros((H, CPACK_COLS), f32)
    for nme, arr in parts.items():
        c0, w_ = CPACK_LAYOUT[nme]
        arr = np.asarray(arr, f32)
        cpack[0:arr.shape[0], c0:c0 + w_] = arr

    in_maps = []
    for c in range(NCORES):
        sl = slice(c * BL, (c + 1) * BL)
        cp = cpack.copy()
        c0, w_ = CPACK_LAYOUT["st"]
        cp[0:2, c0:c0 + w_] = st[sl].transpose(1, 0, 2).reshape(2, BL * S)
        c0, w_ = CPACK_LAYOUT["dy"]
        cp[0:2, c0:c0 + w_] = dy[sl].transpose(1, 0, 2).reshape(2, BL * S)
        in_maps.append({"cpack": cp})
    return in_maps


def kernel(**inputs):
    _ensure_path()
    from concourse import bass_utils

    if "nc" not in _CACHE:
        _CACHE["nc"] = _build_program()
    nc = _CACHE["nc"]

    in_maps = _host_prep(inputs)
    res = bass_utils.run_bass_kernel_spmd(nc, in_maps, core_ids=list(range(NCORES)))
    ptrs = np.concatenate([r["out_idx"] for r in res.results], axis=0)
    logps = np.concatenate([r["out_logp"] for r in res.results], axis=0)
    return ptrs.astype(np.int32), logps.astype(np.float32)
